# revision 53
# baseline (speedup 1.0000x reference)
"""Trainium2 Bass kernel for a GPT-2-style transformer block.

B=1, T=4096, C=768, H=12 heads (hd=64), causal attention, exact GELU MLP.

Distribution over 8 NeuronCores (single shared SPMD program; collectives on
this pool measure ~0.4-1 ms per call, so the design avoids them entirely):
  - Queries: mod-8 interleaved sharding (core c owns tokens t with t%8==c),
    i-DESCENDING within each 1024-token block so causal widths shrink by
    16 cols per key-tile (diagonal tightening of matmul/exp widths).
  - K/V: every core computes the full-sequence K^T/V locally. K^T stays
    SBUF-resident; V is now ALSO SBUF-resident [P, KT, PAIRS, 2, 65] with a
    baked-in ones column per head, so the AV matmul's 65th output row is the
    softmax denominator (the separate ones-matmul row-sum pass is gone; PE
    cost of a matmul depends only on output free size). The reciprocal is
    broadcast back across partitions with a tiny [2,128] sel-matmul.
  - K and V projections run in fp8e4m3 with DoubleRow packing (2x PE rate)
    off a shared packed fp8 copy of LN1(x); Q stays bf16.
  - LN rstd = Rsqrt(var) in ONE activation (the old Ln+Exp pair ping-ponged
    activation tables: the greedy table-loader picks natural_log for Ln and
    exp_and_others for Exp -> 18 x 1.3us reloads).
  - proj/LN2/MLP/residual: row-parallel on each core's own query rows.

All activations live in transposed [feature, token] layout on-chip; LN
statistics use ones-matmul partition reductions; matmul inputs are bf16
(K/V fp8) with fp32 accumulation and an fp32 residual stream.

Hard-won constraints encoded here:
  - one NEFF for all 8 cores (shard_map SPMD) -> no per-core control flow;
  - PSUM is 8 banks: A: qkv(2 tags x 3 bufs=6)+ln(2); B: s(3x2)+y0+y1;
  - exp groups for bands 2-3 pack G k-tiles per [P,2,R] psum tile; tightened
    widths may exp stale psum columns -- consumers only read valid widths;
  - engine split: DVE prefers bf16 sbuf<->sbuf (2x mode), Pool is flat
    0.833ns/row (best for f32), Act engine only does copy/activation with
    per-partition bias.
"""

import numpy as np
import ml_dtypes

import concourse.bacc as bacc
import concourse.mybir as mybir
import concourse.tile as tile
from concourse.bass_utils import run_bass_kernel_spmd

BF16 = ml_dtypes.bfloat16
F8 = ml_dtypes.float8_e4m3

# problem shape (hardcoded per harness contract)
T = 4096
C = 768
H = 12
HD = 64
EPS = 1e-5
NC = 8          # cores
R = 512         # tokens per core
P = 128
CT = C // P     # 6 feature tiles
QT = R // P     # 4 query tiles per core
KT = T // P     # 32 key tiles
PAIRS = H // 2  # 6 head pairs
HT = (4 * C) // P  # 24 hidden tiles

_CACHE = {}
USE_FP8_V = True   # fp8e4m3 + DoubleRow for the replicated V projection too
USE_FP8_Q = True   # fp8e4m3 + DoubleRow for the Q projection
USE_FP8_FC1 = False  # fp8 fc1 costs ~1.3e-2 rel err (no key-averaging in MLP)
USE_PBCAST = False  # partition_broadcast mis-executes on the real backend (NaNs)
TIGHTEN = True     # shrink matmul/exp widths by 16 cols per diagonal k-tile
assert not (USE_FP8_Q and not USE_FP8_V), "Q8+Vbf16 offset combo not wired"


def _ln_transposed(nc, tc, pool, pspool, xT, out_bf, ones_sb, w_col, b_col, apply_wb,
                   x_is_bf16=False):
    """LayerNorm over the feature axis for [C, R]-transposed activations.

    xT: f32 (or bf16 with x_is_bf16) sbuf tile [P, CT, R]; out_bf: bf16 tile.
    Stats via ones-matmul partition reduction (all-partition-broadcast
    results), rstd = Rsqrt(var + eps) in a single activation.
    """
    f32 = mybir.dt.float32
    bf16 = mybir.dt.bfloat16
    if x_is_bf16:
        xb = xT
    else:
        xb = pool.tile([P, CT, R], bf16, tag="ln_xb")
    sq = pool.tile([P, CT, R], bf16, tag="ln_sq")
    for k in range(CT):
        if not x_is_bf16:
            nc.vector.tensor_copy(xb[:, k, :], xT[:, k, :])
        nc.vector.tensor_mul(sq[:, k, :], xb[:, k, :], xb[:, k, :])
    ps_sum = pspool.tile([P, R], f32, tag="ln_psum")
    ps_sq = pspool.tile([P, R], f32, tag="ln_pssq")
    for k in range(CT):
        nc.tensor.matmul(ps_sum[:], ones_sb[:], xb[:, k, :], start=(k == 0), stop=(k == CT - 1))
    for k in range(CT):
        nc.tensor.matmul(ps_sq[:], ones_sb[:], sq[:, k, :], start=(k == 0), stop=(k == CT - 1))
    # gpsimd cannot touch PSUM: psum reads go to Act (Copy w/ scale) and DVE
    nmean = pool.tile([P, R], f32, tag="ln_nmean")
    m2 = pool.tile([P, R], f32, tag="ln_m2")
    AF = mybir.ActivationFunctionType
    nc.scalar.activation(nmean[:], ps_sum[:], AF.Copy, scale=-1.0 / C)
    nc.scalar.activation(m2[:], ps_sq[:], AF.Copy, scale=1.0 / C)
    var = pool.tile([P, R], f32, tag="ln_var")
    nc.gpsimd.tensor_mul(var[:], nmean[:], nmean[:])          # mean^2
    # var = (E[x^2] + eps) - mean^2  (TensorScalarPtr is DVE/Act-only)
    nc.vector.scalar_tensor_tensor(
        var[:], m2[:], EPS, var[:], mybir.AluOpType.add, mybir.AluOpType.subtract
    )
    sd = pool.tile([P, R], f32, tag="ln_sd")
    nc.scalar.activation(sd[:], var[:], AF.Sqrt)
    rstd = pool.tile([P, R], f32, tag="ln_rstd")
    nc.vector.reciprocal(rstd[:], sd[:])
    nmr = pool.tile([P, R], f32, tag="ln_nmr")
    nc.gpsimd.tensor_mul(nmr[:], nmean[:], rstd[:])           # -mu*rstd
    tmp = pool.tile([P, R], f32, tag="ln_tmp")
    tmp2 = pool.tile([P, R], f32, tag="ln_tmp2")
    for k in range(CT):
        # mul on DVE, add on Pool: overlapped engines halve the serial
        # chain latency to the fp8-repack DMA that gates K/V matmuls
        t = tmp if k % 2 == 0 else tmp2
        nc.vector.tensor_mul(t[:], xT[:, k, :], rstd[:])
        if apply_wb:
            nc.gpsimd.tensor_add(t[:], t[:], nmr[:])
            nc.vector.tensor_scalar(
                out_bf[:, k, :], t[:], w_col[:, k : k + 1], b_col[:, k : k + 1],
                mybir.AluOpType.mult, mybir.AluOpType.add,
            )
        else:
            nc.gpsimd.tensor_add(out_bf[:, k, :], t[:], nmr[:])


def _build(apply_ln1, apply_ln2, apply_bv, apply_bqk=False, sim_no_cc=False, reps=1):
    key = (apply_ln1, apply_ln2, apply_bv, apply_bqk, reps)
    if key in _CACHE:
        return _CACHE[key]

    f32 = mybir.dt.float32
    bf16 = mybir.dt.bfloat16
    f8 = mybir.dt.float8e4
    AF = mybir.ActivationFunctionType
    DR = mybir.MatmulPerfMode.DoubleRow

    nc = bacc.Bacc("TRN2", target_bir_lowering=False, debug=False, num_devices=NC)

    xtf_d = nc.declare_dram_parameter("xtf", [C, T], bf16, isOutput=False)
    xTq_d = nc.declare_dram_parameter("xTq", [C, R], f32, isOutput=False)
    masks_d = nc.declare_dram_parameter("masks", [P, 8, P], bf16, isOutput=False)
    ones_d = nc.declare_dram_parameter("ones", [P, P], bf16, isOutput=False)
    if not (USE_FP8_V and USE_FP8_Q):
        WAC = (0 if USE_FP8_Q else C) + (0 if USE_FP8_V else C)
        wattn_d = nc.declare_dram_parameter("wattn", [C, WAC], bf16, isOutput=False)
    if USE_FP8_Q:
        wq8_d = nc.declare_dram_parameter("wq8", [HD, CT, 2, C], f8, isOutput=False)
    wk8_d = nc.declare_dram_parameter("wk8", [HD, CT, 2, C], f8, isOutput=False)
    if USE_FP8_V:
        wv8_d = nc.declare_dram_parameter("wv8", [HD, CT, 2, C], f8, isOutput=False)
    wproj_d = nc.declare_dram_parameter("wproj", [C, C], bf16, isOutput=False)
    if USE_FP8_FC1:
        wfc8_d = nc.declare_dram_parameter("wfc8", [HD, CT, 2, 4 * C], f8, isOutput=False)
    else:
        wfc_d = nc.declare_dram_parameter("wfc", [C, 4 * C], bf16, isOutput=False)
    wfc2_d = nc.declare_dram_parameter("wfc2", [4 * C, C], bf16, isOutput=False)
    bqk_d = nc.declare_dram_parameter("bqk", [P, 2 * CT], f32, isOutput=False)
    bproj_d = nc.declare_dram_parameter("bproj", [P, CT], f32, isOutput=False)
    bfc_d = nc.declare_dram_parameter("bfc", [P, HT], f32, isOutput=False)
    bfc2_d = nc.declare_dram_parameter("bfc2", [P, CT], f32, isOutput=False)
    if apply_bv:
        bv_d = nc.declare_dram_parameter("bv", [P, PAIRS, 2, HD], f32, isOutput=False)
    if apply_ln1:
        ln1w_d = nc.declare_dram_parameter("ln1w", [P, CT], f32, isOutput=False)
        ln1b_d = nc.declare_dram_parameter("ln1b", [P, CT], f32, isOutput=False)
    if apply_ln2:
        ln2w_d = nc.declare_dram_parameter("ln2w", [P, CT], f32, isOutput=False)
        ln2b_d = nc.declare_dram_parameter("ln2b", [P, CT], f32, isOutput=False)
    outT_d = nc.declare_dram_parameter("outT", [C, R], f32, isOutput=True)

    with tile.TileContext(nc) as tc:
        with (
            tc.tile_pool(name="const", bufs=1) as const,
            tc.tile_pool(name="mid", bufs=1) as mid,
        ):
            # startup DMA spread: SP carries only what gates the critical
            # path (ones for stats matmuls, xTq, then slab xv/xp8); weights
            # and biases ride the Pool/Act queues.
            ones_sb = const.tile([P, P], bf16)
            nc.sync.dma_start(ones_sb[:], ones_d[:])
            xTq_sb = const.tile([P, CT, R], f32)
            xTq_r = xTq_d.rearrange("(o p) t -> p o t", p=P)
            nc.sync.dma_start(xTq_sb[:, 0:3, :], xTq_r[:, 0:3, :])
            nc.sync.dma_start(xTq_sb[:, 3:6, :], xTq_r[:, 3:6, :])
            bqk_sb = const.tile([P, 2 * CT], f32)
            nc.gpsimd.dma_start(bqk_sb[:], bqk_d[:])
            bproj_sb = const.tile([P, CT], f32)
            nc.gpsimd.dma_start(bproj_sb[:], bproj_d[:])
            bfc_sb = const.tile([P, HT], f32)
            nc.gpsimd.dma_start(bfc_sb[:], bfc_d[:])
            bfc2_sb = const.tile([P, CT], f32)
            nc.gpsimd.dma_start(bfc2_sb[:], bfc2_d[:])
            if apply_bv:
                bv_sb = const.tile([P, PAIRS, 2, HD], f32)
                nc.gpsimd.dma_start(bv_sb[:], bv_d[:])
            ln1w_sb = ln1b_sb = ln2w_sb = ln2b_sb = None
            if apply_ln1:
                ln1w_sb = const.tile([P, CT], f32)
                ln1b_sb = const.tile([P, CT], f32)
                nc.gpsimd.dma_start(ln1w_sb[:], ln1w_d[:])
                nc.gpsimd.dma_start(ln1b_sb[:], ln1b_d[:])
            if apply_ln2:
                ln2w_sb = const.tile([P, CT], f32)
                ln2b_sb = const.tile([P, CT], f32)
                nc.gpsimd.dma_start(ln2w_sb[:], ln2w_d[:])
                nc.gpsimd.dma_start(ln2b_sb[:], ln2b_d[:])

            # mid-lifetime tiles
            q_sb = mid.tile([P, CT, R], bf16)      # Q^T for own queries
            ynorm_sb = mid.tile([P, CT, R], bf16)  # normalized attn out (y^T)

            for _rep in range(reps):
                # ---------------- Phase A: LN1 + Q + full K/V ----------------
                ktp_cm = tc.tile_pool(name="ktp", bufs=1)
                ktp = ktp_cm.__enter__()
                kt_full = ktp.tile([P, CT, T], bf16)  # resident K^T [C, T]
                # resident V per pair/head with a ones column at feature 64:
                # v_res[:, k, pr, h, 0:64] = V features, [.., 64] = 1.0
                v_res = ktp.tile([P, KT, PAIRS, 2, HD + 1], bf16)
                nc.gpsimd.memset(v_res[:, :, :, :, HD : HD + 1], 1.0)
                with (
                    tc.tile_pool(name="qkvp", bufs=1) as qkvp,
                    tc.tile_pool(name="lnp", bufs=1) as lnp,
                    tc.tile_pool(name="chkp", bufs=2) as chkp,
                    tc.tile_pool(name="ps_ln", bufs=1, space="PSUM") as ps_ln,
                    tc.tile_pool(name="ps_qkv", bufs=3, space="PSUM") as ps_qkv,
                ):
                    wk8_sb = qkvp.tile([HD, CT, 2, C], f8)
                    nc.gpsimd.dma_start(wk8_sb[:], wk8_d[:])
                    if USE_FP8_V:
                        wv8_sb = qkvp.tile([HD, CT, 2, C], f8)
                        nc.gpsimd.dma_start(wv8_sb[:], wv8_d[:])
                    if USE_FP8_Q:
                        wq8_sb = qkvp.tile([HD, CT, 2, C], f8)
                        nc.gpsimd.dma_start(wq8_sb[:], wq8_d[:])
                    if not (USE_FP8_V and USE_FP8_Q):
                        wattn_sb = qkvp.tile([P, CT, WAC], bf16)
                        wattn_r = wattn_d.rearrange("(o p) f -> p o f", p=P)
                        for k in range(CT):
                            nc.scalar.dma_start(wattn_sb[:, k, :], wattn_r[:, k, :])

                    # Q^T for own (interleaved) query rows -- first, so
                    # attention can begin as soon as K/V land
                    if USE_FP8_Q:
                        xq8 = qkvp.tile([P, CT, R], f8)
                        _ln_transposed(nc, tc, lnp, ps_ln, xTq_sb, xq8, ones_sb,
                                       ln1w_sb, ln1b_sb, apply_ln1)
                        xq8p = qkvp.tile([HD, CT, 2, R], f8)
                        for k in range(CT):
                            nc.sync.dma_start(xq8p[:, k, 0, :], xq8[0:HD, k, :])
                            nc.sync.dma_start(xq8p[:, k, 1, :], xq8[HD:P, k, :])
                    else:
                        xln_q = qkvp.tile([P, CT, R], bf16)
                        _ln_transposed(nc, tc, lnp, ps_ln, xTq_sb, xln_q, ones_sb,
                                       ln1w_sb, ln1b_sb, apply_ln1)
                    for f in range(CT):
                        ps = ps_qkv.tile([P, R], f32, tag="qk_ps")
                        for k in range(CT):
                            if USE_FP8_Q:
                                nc.tensor.matmul(
                                    ps[:], wq8_sb[:, k, :, P * f : P * (f + 1)],
                                    xq8p[:, k, :, :], start=(k == 0), stop=(k == CT - 1),
                                    perf_mode=DR,
                                )
                            else:
                                nc.tensor.matmul(
                                    ps[:], wattn_sb[:, k, P * f : P * (f + 1)],
                                    xln_q[:, k, :], start=(k == 0), stop=(k == CT - 1),
                                )
                        if apply_bqk:
                            nc.scalar.activation(
                                q_sb[:, f, :], ps[:], AF.Identity,
                                bias=bqk_sb[:, f : f + 1],
                            )
                        elif f % 2 == 0:
                            nc.scalar.copy(q_sb[:, f, :], ps[:])
                        else:
                            nc.vector.tensor_copy(q_sb[:, f, :], ps[:])

                    VOFS = 0 if USE_FP8_Q else C
                    xtf_r = xtf_d.rearrange("(o p) t -> p o t", p=P)
                    for s in range(NC):
                        xv = chkp.tile([P, CT, R], bf16, tag="xv", name="xv")
                        nc.sync.dma_start(xv[:], xtf_r[:, :, R * s : R * (s + 1)])
                        # LN1 writes fp8 directly (K/V matmuls only consume
                        # the fp8-packed copy), then DMA-repack to DoubleRow
                        # layout [64, CT, 2, R]: contraction row c = j*64+ki.
                        xln8 = chkp.tile([P, CT, R], f8, tag="xln8", name="xln8", bufs=1)
                        if USE_FP8_V:
                            _ln_transposed(nc, tc, lnp, ps_ln, xv, xln8, ones_sb,
                                           ln1w_sb, ln1b_sb, apply_ln1, x_is_bf16=True)
                        else:
                            xln_s = chkp.tile([P, CT, R], bf16, tag="xln_s", name="xln_s")
                            _ln_transposed(nc, tc, lnp, ps_ln, xv, xln_s, ones_sb,
                                           ln1w_sb, ln1b_sb, apply_ln1, x_is_bf16=True)
                            nc.vector.tensor_copy(xln8[0:HD, :, :], xln_s[0:HD, :, :])
                            nc.vector.tensor_copy(xln8[HD:P, :, :], xln_s[HD:P, :, :])
                        xp8 = chkp.tile([HD, CT, 2, R], f8, tag="xp8", name="xp8", bufs=1)
                        # per-k-tile repack DMAs: k=0 lands right after the
                        # k=0 LN write, so the K matmul group starts early
                        for k in range(CT):
                            nc.sync.dma_start(xp8[:, k, 0, :], xln8[0:HD, k, :])
                            nc.sync.dma_start(xp8[:, k, 1, :], xln8[HD:P, k, :])

                        # K^T slab -> straight into the resident K^T buffer
                        for f in range(CT):
                            ps = ps_qkv.tile([P, R], f32, tag="qk_ps")
                            for k in range(CT):
                                nc.tensor.matmul(
                                    ps[:], wk8_sb[:, k, :, P * f : P * (f + 1)],
                                    xp8[:, k, :, :], start=(k == 0), stop=(k == CT - 1),
                                    perf_mode=DR,
                                )
                            dst = kt_full[:, f, R * s : R * (s + 1)]
                            if apply_bqk:
                                nc.scalar.activation(
                                    dst, ps[:], AF.Identity,
                                    bias=bqk_sb[:, CT + f : CT + f + 1],
                                )
                            elif f % 3 == 2:
                                nc.vector.tensor_copy(dst, ps[:])
                            else:
                                nc.scalar.copy(dst, ps[:])

                        # V slab -> resident v_res (natural [token, feature])
                        for t in range(QT):
                            kt_idx = 4 * s + t
                            for hh in range(2):
                                ps = ps_qkv.tile([P, 384], f32, tag="v_ps")
                                for k in range(CT):
                                    if USE_FP8_V:
                                        nc.tensor.matmul(
                                            ps[:], xp8[:, k, :, P * t : P * (t + 1)],
                                            wv8_sb[:, k, :, 384 * hh : 384 * (hh + 1)],
                                            start=(k == 0), stop=(k == CT - 1),
                                            perf_mode=DR,
                                        )
                                    else:
                                        nc.tensor.matmul(
                                            ps[:], xln_s[:, k, P * t : P * (t + 1)],
                                            wattn_sb[:, k, VOFS + 384 * hh : VOFS + 384 * (hh + 1)],
                                            start=(k == 0), stop=(k == CT - 1),
                                        )
                                ps_v = ps[:].rearrange("p (q h d) -> p q h d", q=3, h=2)
                                dst = v_res[:, kt_idx, 3 * hh : 3 * (hh + 1), :, 0:HD]
                                if apply_bv:
                                    nc.vector.tensor_add(
                                        dst, ps_v, bv_sb[:, 3 * hh : 3 * (hh + 1), :, :]
                                    )
                                elif (2 * t + hh) % 2 == 0:
                                    nc.scalar.copy(dst, ps_v)
                                else:
                                    nc.vector.tensor_copy(dst, ps_v)

                # ---------------- Phase B: attention ----------------
                with (
                    tc.tile_pool(name="pp", bufs=3) as pp,
                    tc.tile_pool(name="normp", bufs=2) as normp,
                    tc.tile_pool(name="ps_s", bufs=2, space="PSUM") as ps_s,
                    tc.tile_pool(name="ps_y", bufs=1, space="PSUM") as ps_y,
                ):
                    masks_sb = normp.tile([P, 8, P], bf16, tag="masks", bufs=1)
                    nc.sync.dma_start(masks_sb[:], masks_d[:])
                    for pr in range(PAIRS):
                        y0_ps = ps_y.tile([P, R], f32, tag="y0")
                        y1_ps = ps_y.tile([P, R], f32, tag="y1")
                        for m in range(4):  # bands of 8 key-tiles
                            N = P * (4 - m)
                            # p_band is h-major: [P, head-of-pair, ktile, R]
                            p_band = pp.tile([P, 2, 8, R], bf16, tag="p")
                            # exp-group size: G*N == 512 for bands 2-3
                            G = (1, 1, 2, 4)[m]
                            for g in range(8 // G):
                                s_ps = ps_s.tile([P, 2, R], f32, tag="s", name="s_ps")
                                for dg in range(G):
                                    d = g * G + dg
                                    k = 8 * m + d
                                    Nd = N - 16 * d if TIGHTEN else N
                                    nc.tensor.matmul(
                                        s_ps[:, 0, dg * N : dg * N + Nd],
                                        kt_full[0:HD, pr, P * k : P * (k + 1)],
                                        q_sb[0:HD, pr, 0:Nd],
                                        skip_group_check=True,
                                    )
                                    nc.tensor.matmul(
                                        s_ps[:, 1, dg * N : dg * N + Nd],
                                        kt_full[HD:P, pr, P * k : P * (k + 1)],
                                        q_sb[HD:P, pr, 0:Nd],
                                        skip_group_check=True,
                                    )
                                W = N - 16 * G * g if TIGHTEN else N
                                if G == 1:
                                    nc.scalar.activation(
                                        p_band[:, :, g, :W], s_ps[:, :, :W],
                                        AF.Exp, scale=0.125,
                                    )
                                else:
                                    nc.scalar.activation(
                                        p_band[:, :, g * G : (g + 1) * G, :W],
                                        s_ps[:].rearrange(
                                            "p h (a n) -> p h a n", n=N
                                        )[:, :, :, 0:W],
                                        AF.Exp, scale=0.125,
                                    )
                            for d in range(8):
                                k = 8 * m + d
                                Nd = N - 16 * d if TIGHTEN else N
                                wd = P - 16 * d if TIGHTEN else P
                                # causal mask on this k-tile's diagonal cols
                                meng = nc.vector if d % 2 == 0 else nc.gpsimd
                                meng.tensor_mul(
                                    p_band[:, :, d, N - P : N - P + wd],
                                    p_band[:, :, d, N - P : N - P + wd],
                                    masks_sb[:, d : d + 1, 0:wd].to_broadcast((P, 2, wd)),
                                )
                                nc.tensor.matmul(
                                    y0_ps[0 : HD + 1, 0:Nd], v_res[:, k, pr, 0, :],
                                    p_band[:, 0, d, 0:Nd],
                                    start=(k == 0), stop=(k == KT - 1),
                                    skip_group_check=True,
                                )
                                nc.tensor.matmul(
                                    y1_ps[0 : HD + 1, 0:Nd], v_res[:, k, pr, 1, :],
                                    p_band[:, 1, d, 0:Nd],
                                    start=(k == 0), stop=(k == KT - 1),
                                    skip_group_check=True,
                                )
                        # softmax denominators sit in row HD of y0/y1 (the V
                        # ones-column). Reciprocal -> [2,R], broadcast to all
                        # partitions via sel-matmul, then normalize.
                        rc0 = normp.tile([1, R], bf16, tag="rc0")
                        rc1 = normp.tile([1, R], bf16, tag="rc1")
                        with nc.allow_low_precision(reason="softmax 1/rowsum in bf16"):
                            nc.vector.reciprocal(rc0[:], y0_ps[HD : HD + 1, :])
                            nc.vector.reciprocal(rc1[:], y1_ps[HD : HD + 1, :])
                        # broadcast 1/rowsum across partitions on gpsimd
                        # (no psum, no PE; Pool is idle in phase B)
                        rb_sb = normp.tile([P, R], bf16, tag="rb_sb")
                        if USE_PBCAST:
                            nc.gpsimd.partition_broadcast(rb_sb[0:HD, :], rc0[:])
                            nc.gpsimd.partition_broadcast(rb_sb[HD:P, :], rc1[:])
                        else:
                            rb_ps = ps_s.tile([P, R], f32, tag="rb", name="rb_ps", bufs=1)
                            nc.tensor.matmul(
                                rb_ps[0:HD, :], ones_sb[0:1, 0:HD], rc0[:],
                                skip_group_check=True,
                            )
                            nc.tensor.matmul(
                                rb_ps[HD:P, :], ones_sb[0:1, 0:HD], rc1[:],
                                skip_group_check=True,
                            )
                            nc.vector.tensor_copy(rb_sb[:], rb_ps[:])
                        nc.vector.tensor_mul(
                            ynorm_sb[0:HD, pr, :], y0_ps[0:HD, :], rb_sb[0:HD, :]
                        )
                        nc.vector.tensor_mul(
                            ynorm_sb[HD:P, pr, :], y1_ps[0:HD, :], rb_sb[HD:P, :]
                        )

                ktp_cm.__exit__(None, None, None)

                # ---------------- Phase C: proj + LN2 + MLP + out ----------------
                with (
                    tc.tile_pool(name="mlpp", bufs=1) as mlpp,
                    tc.tile_pool(name="lnp2", bufs=1) as lnp2,
                ):
                    z_sb = mlpp.tile([P, CT, R], f32)      # residual stream x+attn
                    xln2_sb = mlpp.tile([P, CT, R], f8 if USE_FP8_FC1 else bf16)
                    wproj_sb = mlpp.tile([P, CT, C], bf16)
                    wproj_r = wproj_d.rearrange("(o p) f -> p o f", p=P)
                    for k in range(CT):
                        nc.sync.dma_start(wproj_sb[:, k, :], wproj_r[:, k, :])
                    if USE_FP8_FC1:
                        wfc8_sb = mlpp.tile([HD, CT, 2, 4 * C], f8)
                        nc.gpsimd.dma_start(wfc8_sb[:], wfc8_d[:])
                    else:
                        wfc_sb = mlpp.tile([P, CT, 4 * C], bf16)
                        nc.sync.dma_start(wfc_sb[:], wfc_d.rearrange("(o p) f -> p o f", p=P))
                    wfc2_sb = mlpp.tile([P, HT, C], bf16)
                    wfc2_r = wfc2_d.rearrange("(o p) f -> p o f", p=P)
                    for ch in range(4):
                        nc.sync.dma_start(
                            wfc2_sb[:, CT * ch : CT * (ch + 1), :],
                            wfc2_r[:, CT * ch : CT * (ch + 1), :],
                        )
                    with (
                        tc.tile_pool(name="ps_proj", bufs=2, space="PSUM") as ps_proj,
                        tc.tile_pool(name="ps_ln2", bufs=1, space="PSUM") as ps_ln2,
                    ):
                        for f in range(CT):
                            ps = ps_proj.tile([P, R], f32, tag="proj")
                            for k in range(CT):
                                nc.tensor.matmul(
                                    ps[:], wproj_sb[:, k, P * f : P * (f + 1)],
                                    ynorm_sb[:, k, :], start=(k == 0), stop=(k == CT - 1),
                                )
                            # z = (proj + b_proj) + x
                            nc.vector.scalar_tensor_tensor(
                                z_sb[:, f, :], ps[:], bproj_sb[:, f : f + 1], xTq_sb[:, f, :],
                                mybir.AluOpType.add, mybir.AluOpType.add,
                            )
                        _ln_transposed(nc, tc, lnp2, ps_ln2, z_sb, xln2_sb, ones_sb,
                                       ln2w_sb, ln2b_sb, apply_ln2)

                    if USE_FP8_FC1:
                        xc8p = mlpp.tile([HD, CT, 2, R], f8)
                        for k in range(CT):
                            nc.sync.dma_start(xc8p[:, k, 0, :], xln2_sb[0:HD, k, :])
                            nc.sync.dma_start(xc8p[:, k, 1, :], xln2_sb[HD:P, k, :])
                    h_sb = mlpp.tile([P, CT, R], bf16)
                    with (
                        tc.tile_pool(name="ps_fc1", bufs=2, space="PSUM") as ps_fc1,
                        tc.tile_pool(name="ps_o", bufs=1, space="PSUM") as ps_o,
                    ):
                        o_ps = [ps_o.tile([P, R], f32, tag=f"o{f}", name=f"o_ps{f}") for f in range(CT)]
                        for chunk in range(4):
                            for hf in range(CT):
                                hh = CT * chunk + hf
                                ps = ps_fc1.tile([P, R], f32, tag="fc1")
                                for k in range(CT):
                                    if USE_FP8_FC1:
                                        nc.tensor.matmul(
                                            ps[:], wfc8_sb[:, k, :, P * hh : P * (hh + 1)],
                                            xc8p[:, k, :, :],
                                            start=(k == 0), stop=(k == CT - 1),
                                            perf_mode=DR,
                                        )
                                    else:
                                        nc.tensor.matmul(
                                            ps[:], wfc_sb[:, k, P * hh : P * (hh + 1)],
                                            xln2_sb[:, k, :], start=(k == 0), stop=(k == CT - 1),
                                        )
                                nc.scalar.activation(
                                    h_sb[:, hf, :], ps[:], AF.Gelu, bias=bfc_sb[:, hh : hh + 1]
                                )
                                for f in range(CT):
                                    nc.tensor.matmul(
                                        o_ps[f][:], wfc2_sb[:, hh, P * f : P * (f + 1)],
                                        h_sb[:, hf, :], start=(hh == 0), stop=(hh == HT - 1),
                                    )
                        outT_sb = mlpp.tile([P, CT, R], f32)
                        for f in range(CT):
                            nc.vector.scalar_tensor_tensor(
                                outT_sb[:, f, :], o_ps[f][:], bfc2_sb[:, f : f + 1], z_sb[:, f, :],
                                mybir.AluOpType.add, mybir.AluOpType.add,
                            )
                            nc.sync.dma_start(outT_d[P * f : P * (f + 1), :], outT_sb[:, f, :])

    nc.compile()
    _CACHE[key] = nc
    return nc


def _query_tokens(c):
    """Token ids owned by core c, in on-chip column order (j desc, i desc)."""
    return np.concatenate(
        [1024 * j + 8 * np.arange(127, -1, -1) + c for j in (3, 2, 1, 0)]
    )


def _pack_dr(w):
    """[C, F] weight block -> DoubleRow fp8 layout [HD, CT, 2, F]:
    [ki, kk, j, f] = w[kk*128 + j*64 + ki, f]."""
    F = w.shape[1]
    return np.ascontiguousarray(
        w.reshape(CT, 2, HD, F).transpose(2, 0, 1, 3)
    ).astype(F8)


def kernel(x, ln1_w, ln1_b, W_attn, b_attn, W_proj, b_proj,
           ln2_w, ln2_b, W_fc, b_fc, W_fc2, b_fc2):
    x = np.asarray(x, np.float32)
    ln1_w = np.asarray(ln1_w, np.float32)
    ln1_b = np.asarray(ln1_b, np.float32)
    W_attn = np.asarray(W_attn, np.float32)
    b_attn = np.asarray(b_attn, np.float32)
    W_proj = np.asarray(W_proj, np.float32)
    b_proj = np.asarray(b_proj, np.float32)
    ln2_w = np.asarray(ln2_w, np.float32)
    ln2_b = np.asarray(ln2_b, np.float32)
    W_fc = np.asarray(W_fc, np.float32)
    b_fc = np.asarray(b_fc, np.float32)
    W_fc2 = np.asarray(W_fc2, np.float32)
    b_fc2 = np.asarray(b_fc2, np.float32)

    apply_ln1 = not (np.all(ln1_w == 1.0) and np.all(ln1_b == 0.0))
    apply_ln2 = not (np.all(ln2_w == 1.0) and np.all(ln2_b == 0.0))
    apply_bv = bool(np.any(b_attn[2 * C :] != 0.0))
    apply_bqk = bool(np.any(b_attn[: 2 * C] != 0.0))

    nc = _build(apply_ln1, apply_ln2, apply_bv, apply_bqk)

    xf = x[0]  # [T, C]
    wm = {}
    if USE_FP8_Q:
        wm["wq8"] = _pack_dr(W_attn[:, :C])
    if USE_FP8_V:
        wm["wv8"] = _pack_dr(W_attn[:, 2 * C :])
    if not (USE_FP8_Q and USE_FP8_V):
        blocks = ([] if USE_FP8_Q else [W_attn[:, :C]]) + (
            [] if USE_FP8_V else [W_attn[:, 2 * C :]]
        )
        wm["wattn"] = np.ascontiguousarray(
            np.concatenate(blocks, axis=1)
        ).astype(BF16)
    wk8 = _pack_dr(W_attn[:, C : 2 * C])
    wproj_b = W_proj.astype(BF16)
    if USE_FP8_FC1:
        wm["wfc8"] = _pack_dr(W_fc)
    else:
        wm["wfc"] = W_fc.astype(BF16)
    wfc2_b = W_fc2.astype(BF16)
    bqk = np.ascontiguousarray(b_attn[: 2 * C].reshape(2 * CT, P).T)
    bproj = np.ascontiguousarray(b_proj.reshape(CT, P).T)
    bfc = np.ascontiguousarray(b_fc.reshape(HT, P).T)
    bfc2 = np.ascontiguousarray(b_fc2.reshape(CT, P).T)
    ones = np.ones((P, P), BF16)

    xtf = np.ascontiguousarray(xf.T.astype(BF16))
    in_maps = []
    qtok = []
    for c in range(NC):
        qt = _query_tokens(c)
        qtok.append(qt)
        xTq = np.ascontiguousarray(xf[qt, :].T)
        kk = np.arange(P)[:, None, None]
        dd = np.arange(8)[None, :, None]
        pp_ = np.arange(P)[None, None, :]
        # query col p in the diagonal 128-col group is token 8*(127-p)+c
        # within its band; key row kk of k-tile d is token 128*d+kk
        masks = ((8 * (127 - pp_) + c - 128 * dd - kk) >= 0).astype(BF16)
        m = {
            "xtf": xtf, "xTq": xTq, "masks": masks, "ones": ones,
            "wk8": wk8, **wm,
            "wproj": wproj_b, "wfc2": wfc2_b,
            "bqk": bqk, "bproj": bproj, "bfc": bfc, "bfc2": bfc2,
        }
        if apply_bv:
            m["bv"] = np.ascontiguousarray(
                np.broadcast_to(
                    b_attn[2 * C :].reshape(PAIRS, 2, HD), (P, PAIRS, 2, HD)
                )
            )
        if apply_ln1:
            m["ln1w"] = np.ascontiguousarray(ln1_w.reshape(CT, P).T)
            m["ln1b"] = np.ascontiguousarray(ln1_b.reshape(CT, P).T)
        if apply_ln2:
            m["ln2w"] = np.ascontiguousarray(ln2_w.reshape(CT, P).T)
            m["ln2b"] = np.ascontiguousarray(ln2_b.reshape(CT, P).T)
        in_maps.append(m)

    res = run_bass_kernel_spmd(nc, in_maps, list(range(NC)))

    out = np.empty((T, C), np.float32)
    for c in range(NC):
        out[qtok[c], :] = res.results[c]["outT"].T
    return out[None, :, :]
